# revision 20
# baseline (speedup 1.0000x reference)
"""Trainium2 Bass kernel for nn_CustomAttentionLayer (topk_masking).

Computes, per sample b:
    u = x @ W + b              # [T] attention logits
    e = tanh(u)
    a = softmax(e over T)
    top-409 timesteps of a get emphasis x1.5
    out[b] = sum_t x[b,t,:] * a_emph[b,t]      # [1, F]

Strategy (pure data-parallel over batch, 4 samples per core on 8 cores):
  - Stream each sample's x ([4096, 256] f32, 4 MiB) into SBUF once and keep
    it resident (16 MiB/core).  T is mapped to (partition p, lane n) as
    t = 32*p + n so every DMA reads 8 KiB contiguous per partition.
  - u computed by DVE tensor_tensor_reduce (x_tile * W_bcast, sum over F)
    per [128, 256] tile, overlapped with the DMA stream.
  - tanh/exp on ACT.  exp(e - 1) is used: e in [-1, 1] so no max-subtraction
    is needed for softmax stability (matches reference up to fp rounding).
  - Exact top-k threshold via the GPSIMD kth_largest op: it returns the
    410th-largest u exactly; mask = (u > that value) selects exactly the
    top 409 (monotonicity of tanh/softmax makes ranking by u equivalent).
  - Weighted reduction sum_t w_t * x_t on the TensorEngine: 32 chained
    PSUM-accumulating matmuls per sample (lhsT = w column [128,1],
    rhs = x tile [128,256]).
  - Normalize by 1/Z and DMA the [1, 256] row out.
"""

import numpy as np

B, T, F = 32, 4096, 256
N_CORES = 8
SPC = B // N_CORES  # samples per core
NL = T // 128  # lanes per partition (free dim of u)
K = max(1, int(T * 0.1))  # 409
EMPHASIS = 1.5
NQ = 4  # DMA chunks per sample
QN = NL // NQ  # tiles per chunk
QF = QN * F  # chunk free size

# quantile so that k_adj = floor((1-q)*(T-1)) == K-1 = 408; then
# kth_largest's out[0,1] = desc[k_adj+1] = desc[409] = the 410th largest.
QUANT = 1.0 - (K - 0.5) / (T - 1)

_CACHED_NC = None


def _check_quantile():
    omq = max(1, min(int(round((1.0 - QUANT) * 4294967296)), 4294967295))
    k_adj = (omq * (T - 1)) >> 32
    assert k_adj == K - 1, (k_adj, K)


def build_nc(use_f32r=True, skip=(), repeat=1):
    # skip: subset of {"kth", "pbcast", "mm", "ttr"} — debugging aid to
    # bisect hardware hangs; skipped stages are replaced with memsets.
    # repeat: unroll the whole pipeline R times (timing harness only).
    from contextlib import ExitStack

    from concourse import bacc, mybir, tile

    _check_quantile()
    f32 = mybir.dt.float32
    f32r = mybir.dt.float32r
    xdt = f32r if use_f32r else f32
    Alu = mybir.AluOpType
    Act = mybir.ActivationFunctionType

    nc = bacc.Bacc(
        "TRN2",
        target_bir_lowering=False,
        debug=False,
        num_devices=N_CORES,
    )
    x = nc.dram_tensor("x", [SPC, T, F], xdt, kind="ExternalInput").ap()
    W = nc.dram_tensor("W", [F, 1], f32, kind="ExternalInput").ap()
    bvec = nc.dram_tensor("b", [1], f32, kind="ExternalInput").ap()
    y = nc.dram_tensor("y", [SPC, F], f32, kind="ExternalOutput").ap()

    with tile.TileContext(nc) as tc, ExitStack() as ctx:
        const_pool = ctx.enter_context(tc.tile_pool(name="const", bufs=1))
        xpool = ctx.enter_context(tc.tile_pool(name="x", bufs=1))
        spool = ctx.enter_context(tc.tile_pool(name="small", bufs=1))
        scratch = ctx.enter_context(tc.tile_pool(name="scratch", bufs=4))
        ypsum = ctx.enter_context(tc.tile_pool(name="ypsum", bufs=2, space="PSUM"))
        zpsum = ctx.enter_context(tc.tile_pool(name="zpsum", bufs=2, space="PSUM"))

        # --- constants ---
        w_row = const_pool.tile([1, F], f32, tag="w_row")
        nc.sync.dma_start(w_row[:], W.rearrange("f one -> one f"))
        w_bcast = const_pool.tile([128, F], f32, tag="w_bcast")
        b_one = const_pool.tile([1, 1], f32, tag="b_one")
        nc.sync.dma_start(b_one[:], bvec[None, :])
        b_bcast = const_pool.tile([128, 1], f32, tag="b_bcast")
        if "pbcast" in skip:
            nc.vector.memset(w_bcast[:], 0.0625)
            nc.vector.memset(b_bcast[:], 0.0)
        else:
            nc.gpsimd.partition_broadcast(w_bcast[:], w_row[:])
            nc.gpsimd.partition_broadcast(b_bcast[:], b_one[:])

        ones = const_pool.tile([128, 1], f32, tag="ones")
        nc.vector.memset(ones[:], 1.0)

        neg1 = const_pool.tile([128, 1], f32, tag="neg1")
        nc.vector.memset(neg1[:], -1.0)

        for s in [s for _ in range(repeat) for s in range(SPC)]:
            # --- load x[s], resident; t = 32*p + n ---
            xv = x[s].rearrange("(p n) f -> p (n f)", p=128)
            xq = []
            for q in range(NQ):
                xt = xpool.tile([128, QF], xdt, tag=f"x_{s}_{q}")
                nc.sync.dma_start(xt[:], xv[:, q * QF : (q + 1) * QF])
                xq.append(xt)

            # --- u[p, n] = sum_f x[t, f] * W[f],  t = 32p + n ---
            u = spool.tile([128, NL], f32, tag=f"u_{s}")
            if "ttr" in skip:
                nc.vector.memset(u[:], 0.5)
            else:
                for q in range(NQ):
                    for j in range(QN):
                        n = q * QN + j
                        prod = scratch.tile([128, F], f32, tag="prod")
                        nc.vector.scalar_tensor_tensor(
                            out=prod[:],
                            in0=xq[q][:, j * F : (j + 1) * F].bitcast(f32),
                            scalar=1.0,
                            in1=w_bcast[:],
                            op0=Alu.mult,
                            op1=Alu.mult,
                            accum_out=u[:, n : n + 1],
                        )

            # --- e = tanh(u + b); p = exp(e - 1); zpart = sum_n p ---
            e = spool.tile([128, NL], f32, tag=f"e_{s}")
            nc.scalar.activation(e[:], u[:], Act.Tanh, bias=b_bcast[:])
            p_ = spool.tile([128, NL], f32, tag=f"p_{s}")
            zpart = spool.tile([128, 1], f32, tag=f"zp_{s}")
            nc.scalar.activation(p_[:], e[:], Act.Exp, bias=neg1[:], accum_out=zpart[:])

            # --- Z = sum(zpart) via PE; zinv = 1/Z ---
            zps = zpsum.tile([1, 1], f32, tag="zps")
            nc.tensor.matmul(zps[:], lhsT=zpart[:], rhs=ones[:], start=True, stop=True)
            zinv = spool.tile([1, 1], f32, tag=f"zi_{s}")
            nc.vector.reciprocal(zinv[:], zps[:])

            # --- exact top-k threshold: theta = 410th largest u ---
            kth = spool.tile([1, 2], f32, tag=f"kth_{s}")
            thb = spool.tile([128, 1], f32, tag=f"th_{s}")
            if "kth" in skip:
                nc.vector.memset(kth[:], 1e30)
                nc.vector.memset(thb[:], 1e30)
            else:
                nc.gpsimd.kth_largest(
                    kth[:], u[:], n_per_lane=NL, k=K + 1, quantile=QUANT
                )
                if "pbcast" in skip:
                    nc.vector.memset(thb[:], 1e30)
                else:
                    nc.gpsimd.partition_broadcast(thb[:], kth[0:1, 1:2])

            # --- w = p * (1 + 0.5 * (u > theta)) ---
            c = spool.tile([128, NL], f32, tag=f"c_{s}")
            nc.vector.tensor_scalar(
                out=c[:], in0=u[:], scalar1=thb[:], scalar2=EMPHASIS - 1.0,
                op0=Alu.is_gt, op1=Alu.mult,
            )
            wgt = spool.tile([128, NL], xdt, tag=f"w_{s}")
            nc.vector.scalar_tensor_tensor(
                out=wgt[:], in0=c[:], scalar=1.0, in1=p_[:],
                op0=Alu.add, op1=Alu.mult,
            )

            # --- out = sum_t w_t * x_t  (PE, PSUM-accumulate) ---
            ysb = spool.tile([1, F], f32, tag=f"y_{s}")
            if "mm" in skip:
                nc.vector.memset(ysb[:], 0.0)
            else:
                yps = ypsum.tile([1, F], f32, tag="yps")
                for q in range(NQ):
                    for j in range(QN):
                        n = q * QN + j
                        nc.tensor.matmul(
                            yps[:],
                            lhsT=wgt[:, n : n + 1],
                            rhs=xq[q][:, j * F : (j + 1) * F],
                            start=(n == 0),
                            stop=(n == NL - 1),
                        )
                # --- normalize and store ---
                nc.vector.tensor_scalar_mul(ysb[:], yps[:], zinv[:])
            nc.sync.dma_start(y[s][None, :], ysb[:])

    nc.compile()
    return nc


def _get_nc():
    global _CACHED_NC
    if _CACHED_NC is None:
        _CACHED_NC = build_nc()
    return _CACHED_NC


def make_in_maps(x, W, b):
    x = np.ascontiguousarray(np.asarray(x, dtype=np.float32))
    W = np.ascontiguousarray(np.asarray(W, dtype=np.float32))
    b = np.ascontiguousarray(np.asarray(b, dtype=np.float32))
    return [
        {"x": x[c * SPC : (c + 1) * SPC], "W": W, "b": b} for c in range(N_CORES)
    ]


def kernel(**inputs):
    from concourse.bass_utils import run_bass_kernel_spmd

    nc = _get_nc()
    in_maps = make_in_maps(inputs["x"], inputs["W"], inputs["b"])
    res = run_bass_kernel_spmd(nc, in_maps, core_ids=list(range(N_CORES)))
    ys = [res.results[c]["y"] for c in range(N_CORES)]
    return np.concatenate(ys, axis=0).reshape(B, 1, F).astype(np.float32)
